# revision 29
# baseline (speedup 1.0000x reference)
"""Trainium2 Bass kernel for nn_MemoryAugmented (scatter_memory).

Computes, for full inputs x:[64,12,883,64], M:[12,64,64]:
    score = softmax(einsum('blnd,tmd->btnm', x, M), axis=-1)
    out   = einsum('btnm,tmd->btnd', score, M)

Distribution: data-parallel over batch across 8 NeuronCores (8 batches
per core); the memory bank M is replicated, shipped pre-transformed into
two constant matrices (paired-t M^T for mm1, block-diagonal M + ones
columns for mm2's fused row sums).

Precision: x and M travel as fp16 (matmuls run at 1 cycle/row vs 4 for
fp32, HBM traffic halves); exp values are bf16 (need fp32-like range);
PSUM accumulation is always fp32; output is stored fp16 and upcast on
the host. Measured end-to-end max rel err ~4e-3 vs the 2e-2 gate.

Per-core dataflow, 7 iterations of 1024 rows r = (b, n), software-
pipelined one deep so no engine waits on another's latest result.
Each body iterates 6 t-pair slots, each issuing: one or two 128-row
value chunks of mm2(it-1) + reciprocal + normalize (5 chunks on DVE,
3 via ACT-evacuate + gpsimd), then mm1(it) x2 + one merged exp(it).
The l-sum tree for it+1 (L1/L4 on DVE at fp16 2x, L2/L3 on gpsimd),
the two 0.75 MB store halves of it-1, and the PE transposes + DVE
copy building xsT(it+1) are woven into fixed slots. All PSUM rides
one shared 4-buffer pool of [128,1024] tiles (8 banks); no matmul
output ever crosses a 2 KB PSUM bank boundary (silent corruption).
"""
import sys

for _p in ("/opt/trn_rl_repo",):
    if _p not in sys.path:
        sys.path.insert(0, _p)

from contextlib import ExitStack

import numpy as np

import concourse.bass as bass
import concourse.bacc as bacc
import concourse.tile as tile
from concourse import mybir
from concourse._compat import with_exitstack
from concourse.bass_utils import run_bass_kernel_spmd

B, L, N, D = 64, 12, 883, 64
T, MNUM = 12, 64
NCORES = 8
BS = B // NCORES          # 8 batches per core
NPAD = 896                # per-batch row pad (7*128)
ROWS = BS * NPAD          # 7168 rows per core
NIT = 7                   # iterations of 1024 rows
F32 = mybir.dt.float32
F16 = mybir.dt.float16
BF16 = mybir.dt.bfloat16


def build_consts(M):
    """Host-side layout prep (pure data movement) of the memory bank."""
    M = np.asarray(M, dtype=np.float32)
    mt2h = np.zeros((64, 6 * 128), np.float16)   # [d, (tp, q, m)] = M[2tp+q].T
    mbd = np.zeros((128, 6 * 130), np.float32)   # [(q, m), (tp, q, d | sums)]
    for tp in range(6):
        t0, t1 = 2 * tp, 2 * tp + 1
        mt2h[:, tp * 128 + 0:tp * 128 + 64] = M[t0].T.astype(np.float16)
        mt2h[:, tp * 128 + 64:tp * 128 + 128] = M[t1].T.astype(np.float16)
        mbd[0:64, tp * 130 + 0:tp * 130 + 64] = M[t0]
        mbd[64:128, tp * 130 + 64:tp * 130 + 128] = M[t1]
        mbd[0:64, tp * 130 + 128] = 1.0
        mbd[64:128, tp * 130 + 129] = 1.0
    # mirrored into both partition halves: parity-1 matmuls read their
    # stationary from partitions 64:128 (row group h1)
    mt2h2 = np.concatenate([mt2h, mt2h], axis=0)
    eye = np.eye(128, dtype=np.float32)
    return mt2h2, mbd, eye


@with_exitstack
def kernel_body(ctx: ExitStack, tc: "tile.TileContext", out: bass.AP,
                x: bass.AP, mt2h: bass.AP, mbd: bass.AP, eye: bass.AP):
    nc = tc.nc
    consts = ctx.enter_context(tc.tile_pool(name="consts", bufs=1))
    work = ctx.enter_context(tc.tile_pool(name="work", bufs=2))
    psum = ctx.enter_context(tc.tile_pool(name="psum", bufs=1, space="PSUM"))

    # const loads ride the scalar HWDGE ring (idle at kernel start) so the
    # first x-load isn't queued behind them on the sync ring's FIFO.
    mt2h_sb = consts.tile([128, 6 * 128], F16)
    nc.scalar.dma_start(out=mt2h_sb[:], in_=mt2h[:])
    mbd_sb = consts.tile([128, 6 * 130], BF16)
    nc.scalar.dma_start(out=mbd_sb[:], in_=mbd[:])
    eye_sb = consts.tile([128, 128], F32)
    nc.scalar.dma_start(out=eye_sb[:], in_=eye[:])

    def load(it):
        # one 1.5 MB load; partition p <- rows 8p..8p+7 (12 KB contiguous)
        xt = work.tile([128, 8 * L * D], F16, tag="xt", bufs=3)
        nc.sync.dma_start(
            out=xt[:].rearrange("p (c f) -> p c f", c=8),
            in_=x[1024 * it:1024 * it + 1024, :, :]
                .rearrange("(p c) l d -> p c (l d)", c=8),
        )
        return xt

    def tree_l1(xt):
        # 12 -> 6 on DVE: fp16 tensor_tensor runs 2x there (~1.8us) vs
        # gpsimd's measured ~0.5 elem/cycle (~6us for the same add)
        t384 = work.tile([128, 8 * 384], F16, tag="t384", bufs=3)
        xtv = xt[:].rearrange("p (c h f) -> p c h f", c=8, h=2)
        nc.vector.tensor_add(t384[:].rearrange("p (c f) -> p c f", c=8),
                             xtv[:, :, 0], xtv[:, :, 1])
        return t384

    def tree_rest(t384):
        # L2/L3 on gpsimd, L4 on DVE (DVE is the busier engine)
        t192 = work.tile([128, 8 * 192], F16, tag="t192", bufs=2)
        t384v = t384[:].rearrange("p (c h f) -> p c h f", c=8, h=2)
        nc.gpsimd.tensor_add(t192[:].rearrange("p (c f) -> p c f", c=8),
                             t384v[:, :, 0], t384v[:, :, 1])
        t192v = t192[:].rearrange("p (c g f) -> p c g f", c=8, g=3)
        xs2 = work.tile([128, 8 * 64], F16, tag="xs2", bufs=2)
        xs2v = xs2[:].rearrange("p (c f) -> p c f", c=8)
        nc.gpsimd.tensor_add(xs2v, t192v[:, :, 0], t192v[:, :, 1])
        # fp32 to match the PE transpose's fp32 "val" PSUM staging tile
        xs4 = work.tile([128, 8 * 64], F32, tag="xs4", bufs=2)
        nc.vector.tensor_add(xs4[:].rearrange("p (c f) -> p c f", c=8),
                             xs2v, t192v[:, :, 2])
        return xs4

    def build_xsT(xs4):
        # paired transpose: each [128, 128] block holds chunks 2q (rows
        # 0:64) and 2q+1 (rows 64:128) -> xsT2 [128, (q, n)] via one copy.
        # The PSUM staging borrows a "val"-tag buffer.
        ps_x = psum.tile([128, 1024], F32, tag="ps", bufs=4)
        for q in range(4):
            nc.tensor.transpose(ps_x[:, q * 128:(q + 1) * 128],
                                xs4[:, q * 128:(q + 1) * 128], eye_sb[:])
        xsT2 = work.tile([128, 512], F16, tag="xsT", bufs=3)
        # DVE (not ACT) evacuates: ACT is the pacing engine at ~95% busy
        nc.vector.tensor_copy(xsT2[:], ps_x[:, 0:512])
        return xsT2

    def mm1_exp(xsT2, tp):
        # both parities' logits land in one 2-bank PSUM tile so a single
        # ACT exp covers 1024 elems (amortizes the ~352-cycle ACT fixed cost)
        ps_log = psum.tile([128, 1024], F32, tag="ps", bufs=4)
        for par in range(2):
            b = 64 * par
            nc.tensor.matmul(ps_log[:, 512 * par:512 * par + 512],
                             mt2h_sb[b:b + 64, tp * 128:(tp + 1) * 128],
                             xsT2[b:b + 64, :], start=True, stop=True)
        ex = work.tile([128, 1024], BF16, tag="exp", bufs=12)
        nc.scalar.activation(ex[:], ps_log[:],
                             mybir.ActivationFunctionType.Exp)
        return ex

    def chunk_mm2(exps, c):
        # chunk c lives at parity c%2, block c//2 of the exp tiles.
        # tp blocks land contiguously at 122+130*tp so the whole chunk's
        # normalize is ONE 4D-AP instruction (amortizes DVE/gpsimd fixed
        # cost) and the sums sit at a regular [p, 6, 2] stride pattern.
        q, par = divmod(c, 2)
        ps_val = psum.tile([128, 1024], F32, tag="ps", bufs=4)
        # heartbeat into dead columns (chunks use 122:902): a cheap fp16
        # matmul on chunks 0/4 nudges the PE HAM activity monitor away
        # from re-throttling the array clock to 1.2 GHz.
        if c in (0, 4):
            nc.tensor.matmul(ps_val[:, 0:122], mt2h_sb[0:64, 0:128],
                             mt2h_sb[0:64, 0:122], start=True, stop=True)
        # base offset 122: block 2 ends exactly at col 512 and block 3
        # starts there, so no single matmul output straddles a PSUM bank
        # boundary (straddling corrupts a few boundary elements, timing-
        # dependent) while keeping one uniform 130-stride for the APs.
        for tp in range(6):
            nc.tensor.matmul(ps_val[:, 122 + 130 * tp:122 + 130 * tp + 130],
                             exps[tp][:, 512 * par + q * 128:
                                       512 * par + (q + 1) * 128],
                             mbd_sb[:, tp * 130:(tp + 1) * 130],
                             start=True, stop=True)
        sums_ap = (ps_val[:, 122:902].rearrange("p (a r) -> p a r", a=6)
                   [:, :, 128:130])
        rec = work.tile([128, 12], F32, tag="rec", bufs=8)
        nc.vector.reciprocal(
            rec[:].rearrange("p (a t) -> p a t", a=6), sums_ap)
        return ps_val, rec

    def _norm_aps(ps_or_vv, rec, vn, c, from_psum):
        if from_psum:
            in0 = (ps_or_vv[:, 122:902].rearrange("p (a r) -> p a r", a=6)
                   [:, :, 0:128].rearrange("p a (t d) -> p a t d", t=2))
        else:
            in0 = ps_or_vv[:].rearrange("p (a t d) -> p a t d", a=6, t=2)
        in1 = (rec[:].rearrange("p (a t) -> p a t", a=6)
               .unsqueeze(3).broadcast_to([128, 6, 2, D]))
        outp = (vn[:, c * 768:(c + 1) * 768]
                .rearrange("p (a t d) -> p a t d", a=6, t=2))
        return in0, in1, outp

    def chunk_norm(ps_val, rec, vn, c):
        in0, in1, outp = _norm_aps(ps_val, rec, vn, c, True)
        nc.vector.tensor_mul(outp, in0, in1)

    def chunk_norm_off(ps_val, rec, vn, c):
        # offloaded normalize: ACT evacuates PSUM -> bf16 SBUF in one copy
        # (unnormalized values can reach ~e^30: needs bf16 range), gpsimd
        # (otherwise idle) does the broadcast multiply, freeing DVE.
        vv = work.tile([128, 768], BF16, tag="vv", bufs=3)
        nc.scalar.copy(vv[:].rearrange("p (a r) -> p a r", a=6),
                       ps_val[:, 122:902].rearrange("p (a r) -> p a r", a=6)
                       [:, :, 0:128])
        in0, in1, outp = _norm_aps(vv, rec, vn, c, False)
        nc.gpsimd.tensor_mul(outp, in0, in1)

    def store(it, vn, half):
        # 0.75 MB half-stores: chunk k lives in vn column-group k (row
        # 8p+k), so half h = column-groups 4h..4h+3 = chunks 4h..4h+3.
        # Chunks 0-3 norm by slot 3, chunk 4 (with 5-7) by slot 4; issuing
        # each half as soon as it's ready smooths the DMA stream and trims
        # the epilogue tail. HBM side stays 6 KB-contiguous per partition.
        nc.sync.dma_start(
            out=out[1024 * it:1024 * it + 1024, :]
                .rearrange("(p c) f -> p c f", c=8)[:, 4 * half:4 * half + 4],
            in_=vn[:, 3072 * half:3072 * half + 3072]
                .rearrange("p (c f) -> p c f", c=4),
        )

    # -------- prologue: iteration 0's xsT, loads for 0 and 1 --------
    # iteration 0's load + tree + transpose chain is quartered (2 of the
    # 8 row-groups per step) so its tree starts ~3us after the first
    # quarter-DMA lands instead of waiting out the full 1.5 MB load; the
    # fill phase to the first store is ~1/3 of total runtime. fp16 (not
    # fp32) warm-up matmuls: an fp32 matmul in HIGH mode disables the
    # compiler's fast-weight-load for what follows, and the short burst
    # fits the PE-queue idle window before the first transpose arrives.
    xt0 = work.tile([128, 8 * L * D], F16, tag="xt", bufs=3)
    xt0v = xt0[:].rearrange("p (c f) -> p c f", c=8)
    xsrc = x[0:1024, :, :].rearrange("(p c) l d -> p c (l d)", c=8)
    for qt in range(4):
        nc.sync.dma_start(out=xt0v[:, 2 * qt:2 * qt + 2],
                          in_=xsrc[:, 2 * qt:2 * qt + 2])
    xts = {0: xt0}
    if NIT > 1:
        xts[1] = load(1)
    warm = psum.tile([128, 1024], F32, tag="ps", bufs=4)
    for _ in range(4):
        nc.tensor.matmul(warm[:, 0:512], mt2h_sb[0:64, 0:128],
                         mt2h_sb[0:64, 0:512], start=True, stop=True)
    t384_0 = work.tile([128, 8 * 384], F16, tag="t384", bufs=3)
    t192_0 = work.tile([128, 8 * 192], F16, tag="t192", bufs=2)
    xs2_0 = work.tile([128, 8 * 64], F16, tag="xs2", bufs=2)
    xs4_0 = work.tile([128, 8 * 64], F32, tag="xs4", bufs=2)
    ps_x0 = psum.tile([128, 1024], F32, tag="ps", bufs=4)
    for qt in range(4):
        sl = slice(2 * qt, 2 * qt + 2)
        xtv = xt0[:].rearrange("p (c h f) -> p c h f", c=8, h=2)[:, sl]
        nc.vector.tensor_add(
            t384_0[:].rearrange("p (c f) -> p c f", c=8)[:, sl],
            xtv[:, :, 0], xtv[:, :, 1])
        t384v = t384_0[:].rearrange("p (c h f) -> p c h f", c=8, h=2)[:, sl]
        nc.gpsimd.tensor_add(
            t192_0[:].rearrange("p (c f) -> p c f", c=8)[:, sl],
            t384v[:, :, 0], t384v[:, :, 1])
        t192v = t192_0[:].rearrange("p (c g f) -> p c g f", c=8, g=3)[:, sl]
        xs2v = xs2_0[:].rearrange("p (c f) -> p c f", c=8)[:, sl]
        nc.gpsimd.tensor_add(xs2v, t192v[:, :, 0], t192v[:, :, 1])
        nc.vector.tensor_add(
            xs4_0[:].rearrange("p (c f) -> p c f", c=8)[:, sl],
            xs2v, t192v[:, :, 2])
        nc.tensor.transpose(ps_x0[:, qt * 128:(qt + 1) * 128],
                            xs4_0[:, qt * 128:(qt + 1) * 128], eye_sb[:])
    xsT2 = work.tile([128, 512], F16, tag="xsT", bufs=3)
    nc.vector.tensor_copy(xsT2[:], ps_x0[:, 0:512])

    exps_prev = None
    vn_prev = None
    for it in range(NIT + 1):
        if it + 2 < NIT:
            xts[it + 2] = load(it + 2)
        exps = {}
        vn = None
        if it < NIT:
            vn = work.tile([128, 8 * T * D], F16, tag="vn", bufs=3)
        t384n = None
        xs4n = None
        xsT2_next = None
        # interleave this iteration's mm1/exp pairs with the previous
        # iteration's mm2 chunks so PE never idles on ACT's exp pace; the
        # next iteration's tree and transposes are woven in mid-body.
        # chunk schedule: all 8 chunks spread across the 6 tp slots so
        # every engine stays co-busy through the body (a dedicated chunk
        # tail was tried and regressed 15%: it serializes engine phases).
        # gpsimd-offloaded chunks (5-7) pair with DVE chunks mid-body so
        # their ACT evacuation copies land between exps.
        slot_chunks = {0: [(0, False)], 1: [(5, True), (1, False)],
                       2: [(6, True), (2, False)], 3: [(7, True), (3, False)],
                       4: [(4, False)], 5: []}
        for tp in range(6):
            # previous iteration's chunks first: their inputs are always
            # ready, so they hide the mm1->exp ps_log wait and keep PE
            # dense (HAM stays at full clock).
            work_items = []
            if it > 0:
                for c, off in slot_chunks[tp]:
                    pv, rec = chunk_mm2(exps_prev, c)
                    work_items.append((c, off, pv, rec))
            if it < NIT:
                exps[tp] = mm1_exp(xsT2, tp)
            for c, off, pv, rec in work_items:
                if off:
                    chunk_norm_off(pv, rec, vn_prev, c)
                else:
                    chunk_norm(pv, rec, vn_prev, c)
            if tp == 0 and it + 1 < NIT:
                t384n = tree_l1(xts.pop(it + 1))
            if tp == 1 and t384n is not None:
                xs4n = tree_rest(t384n)
            if tp == 3 and it > 0:
                store(it - 1, vn_prev, 0)
            if tp == 4 and it > 0:
                store(it - 1, vn_prev, 1)
            if tp == 5 and xs4n is not None:
                xsT2_next = build_xsT(xs4n)
        if xsT2_next is not None:
            xsT2 = xsT2_next
        exps_prev, vn_prev = exps, vn


_NC_CACHE = {}


def build_nc():
    if "nc" in _NC_CACHE:
        return _NC_CACHE["nc"]
    nc = bacc.Bacc("TRN2", target_bir_lowering=False, debug=False,
                   num_devices=NCORES)
    # x is pre-transposed on the host to [BS, N, L, D], n-padded to 896 rows
    # per batch with zeros, flattened to [7168, 12, 64] and cast fp16. The
    # output is produced padded as [7168, (t d)] fp16; the host slices off
    # the 13 pad rows per batch and upcasts.
    x_ap = nc.dram_tensor("x_sh", [ROWS, L, D], F16, kind="ExternalInput").ap()
    mt2h_ap = nc.dram_tensor("mt2h", [128, 6 * 128], F16, kind="ExternalInput").ap()
    mbd_ap = nc.dram_tensor("mbd", [128, 6 * 130], BF16, kind="ExternalInput").ap()
    eye_ap = nc.dram_tensor("eye", [128, 128], F32, kind="ExternalInput").ap()
    out_ap = nc.dram_tensor("out", [ROWS, T * D], F16, kind="ExternalOutput").ap()
    with tile.TileContext(nc) as tc:
        kernel_body(tc, out_ap, x_ap, mt2h_ap, mbd_ap, eye_ap)
    nc.compile()
    _NC_CACHE["nc"] = nc
    return nc


def make_in_maps(x, M):
    import ml_dtypes
    x = np.asarray(x, dtype=np.float32)
    mt2h, mbd, eye = build_consts(M)
    mbd_bf = mbd.astype(ml_dtypes.bfloat16)
    maps = []
    for i in range(NCORES):
        xp = np.zeros((BS, NPAD, L, D), np.float16)
        xp[:, :N] = x[i * BS:(i + 1) * BS].transpose(0, 2, 1, 3).astype(np.float16)
        maps.append({"x_sh": xp.reshape(ROWS, L, D),
                     "mt2h": mt2h, "mbd": mbd_bf, "eye": eye})
    return maps


def gather_outputs(res):
    outs = []
    for i in range(NCORES):
        o = np.asarray(res[i]["out"], dtype=np.float32)
        o = o.reshape(BS, NPAD, T, D)[:, :N].transpose(0, 2, 1, 3)
        outs.append(o)
    return np.ascontiguousarray(np.concatenate(outs, axis=0))


def kernel(x, M):
    nc = build_nc()
    in_maps = make_in_maps(x, M)
    res = run_bass_kernel_spmd(nc, in_maps, list(range(NCORES))).results
    return gather_outputs(res)


if __name__ == "__main__":
    rng = np.random.default_rng(0)
    x = rng.standard_normal((B, L, N, D), dtype=np.float32)
    M = (rng.standard_normal((T, MNUM, D), dtype=np.float32) * 0.125).astype(np.float32)
    out = kernel(x, M)
    print("out", out.shape, out.dtype, float(np.abs(out).max()))



# revision 30
# speedup vs baseline: 1.1531x; 1.1531x over previous
"""Trainium2 Bass kernel for nn_MemoryAugmented (scatter_memory).

Computes, for full inputs x:[64,12,883,64], M:[12,64,64]:
    score = softmax(einsum('blnd,tmd->btnm', x, M), axis=-1)
    out   = einsum('btnm,tmd->btnd', score, M)

Distribution: data-parallel over batch across 8 NeuronCores (8 batches
per core); the memory bank M is replicated, shipped pre-transformed into
two constant matrices (paired-t M^T for mm1, block-diagonal M + ones
columns for mm2's fused row sums).

Precision: x and M travel as fp16 (matmuls run at 1 cycle/row vs 4 for
fp32, HBM traffic halves); exp values are bf16 (need fp32-like range);
PSUM accumulation is always fp32; output is stored fp16 and upcast on
the host. Measured end-to-end max rel err ~4e-3 vs the 2e-2 gate.

Per-core dataflow, 7 iterations of 1024 rows r = (b, n), software-
pipelined one deep so no engine waits on another's latest result.
Each body iterates 6 t-pair slots, each issuing: one or two 128-row
value chunks of mm2(it-1) + reciprocal + normalize (5 chunks on DVE,
3 via ACT-evacuate + gpsimd), then mm1(it) x2 + one merged exp(it).
The l-sum tree for it+1 (L1/L4 on DVE at fp16 2x, L2/L3 on gpsimd),
the two 0.75 MB store halves of it-1, and the PE transposes + DVE
copy building xsT(it+1) are woven into fixed slots. All PSUM rides
one shared 4-buffer pool of [128,1024] tiles (8 banks); no matmul
output ever crosses a 2 KB PSUM bank boundary (silent corruption).
"""
import sys

for _p in ("/opt/trn_rl_repo",):
    if _p not in sys.path:
        sys.path.insert(0, _p)

from contextlib import ExitStack

import numpy as np

import concourse.bass as bass
import concourse.bacc as bacc
import concourse.tile as tile
from concourse import mybir
from concourse._compat import with_exitstack
from concourse.bass_utils import run_bass_kernel_spmd

B, L, N, D = 64, 12, 883, 64
T, MNUM = 12, 64
NCORES = 8
BS = B // NCORES          # 8 batches per core
NPAD = 896                # per-batch row pad (7*128)
ROWS = BS * NPAD          # 7168 rows per core
NIT = 7                   # iterations of 1024 rows
F32 = mybir.dt.float32
F16 = mybir.dt.float16
BF16 = mybir.dt.bfloat16


def build_consts(M):
    """Host-side layout prep (pure data movement) of the memory bank."""
    M = np.asarray(M, dtype=np.float32)
    mt2h = np.zeros((64, 6 * 128), np.float16)   # [d, (tp, q, m)] = M[2tp+q].T
    mbd = np.zeros((128, 6 * 130), np.float32)   # [(q, m), (tp, q, d | sums)]
    for tp in range(6):
        t0, t1 = 2 * tp, 2 * tp + 1
        mt2h[:, tp * 128 + 0:tp * 128 + 64] = M[t0].T.astype(np.float16)
        mt2h[:, tp * 128 + 64:tp * 128 + 128] = M[t1].T.astype(np.float16)
        mbd[0:64, tp * 130 + 0:tp * 130 + 64] = M[t0]
        mbd[64:128, tp * 130 + 64:tp * 130 + 128] = M[t1]
        mbd[0:64, tp * 130 + 128] = 1.0
        mbd[64:128, tp * 130 + 129] = 1.0
    # mirrored into both partition halves: parity-1 matmuls read their
    # stationary from partitions 64:128 (row group h1)
    mt2h2 = np.concatenate([mt2h, mt2h], axis=0)
    eye = np.eye(128, dtype=np.float32)
    return mt2h2, mbd, eye


@with_exitstack
def kernel_body(ctx: ExitStack, tc: "tile.TileContext", out: bass.AP,
                x: bass.AP, mt2h: bass.AP, mbd: bass.AP, eye: bass.AP):
    nc = tc.nc
    consts = ctx.enter_context(tc.tile_pool(name="consts", bufs=1))
    work = ctx.enter_context(tc.tile_pool(name="work", bufs=2))
    psum = ctx.enter_context(tc.tile_pool(name="psum", bufs=1, space="PSUM"))

    # const loads ride the scalar HWDGE ring (idle at kernel start) so the
    # first x-load isn't queued behind them on the sync ring's FIFO.
    mt2h_sb = consts.tile([128, 6 * 128], F16)
    nc.scalar.dma_start(out=mt2h_sb[:], in_=mt2h[:])
    mbd_sb = consts.tile([128, 6 * 130], BF16)
    nc.scalar.dma_start(out=mbd_sb[:], in_=mbd[:])
    eye_sb = consts.tile([128, 128], F32)
    nc.scalar.dma_start(out=eye_sb[:], in_=eye[:])

    def load(it):
        # one 1.5 MB load; partition p <- rows 8p..8p+7 (12 KB contiguous)
        xt = work.tile([128, 8 * L * D], F16, tag="xt", bufs=3)
        nc.sync.dma_start(
            out=xt[:].rearrange("p (c f) -> p c f", c=8),
            in_=x[1024 * it:1024 * it + 1024, :, :]
                .rearrange("(p c) l d -> p c (l d)", c=8),
        )
        return xt

    def tree_l1(xt):
        # 12 -> 6 on DVE: fp16 tensor_tensor runs 2x there (~1.8us) vs
        # gpsimd's measured ~0.5 elem/cycle (~6us for the same add)
        t384 = work.tile([128, 8 * 384], F16, tag="t384", bufs=3)
        xtv = xt[:].rearrange("p (c h f) -> p c h f", c=8, h=2)
        nc.vector.tensor_add(t384[:].rearrange("p (c f) -> p c f", c=8),
                             xtv[:, :, 0], xtv[:, :, 1])
        return t384

    def tree_rest(t384):
        # L2/L3 on gpsimd, L4 on DVE (DVE is the busier engine)
        t192 = work.tile([128, 8 * 192], F16, tag="t192", bufs=2)
        t384v = t384[:].rearrange("p (c h f) -> p c h f", c=8, h=2)
        nc.gpsimd.tensor_add(t192[:].rearrange("p (c f) -> p c f", c=8),
                             t384v[:, :, 0], t384v[:, :, 1])
        t192v = t192[:].rearrange("p (c g f) -> p c g f", c=8, g=3)
        xs2 = work.tile([128, 8 * 64], F16, tag="xs2", bufs=2)
        xs2v = xs2[:].rearrange("p (c f) -> p c f", c=8)
        nc.gpsimd.tensor_add(xs2v, t192v[:, :, 0], t192v[:, :, 1])
        # fp32 to match the PE transpose's fp32 "val" PSUM staging tile
        xs4 = work.tile([128, 8 * 64], F32, tag="xs4", bufs=2)
        nc.vector.tensor_add(xs4[:].rearrange("p (c f) -> p c f", c=8),
                             xs2v, t192v[:, :, 2])
        return xs4

    def build_xsT(xs4):
        # paired transpose: each [128, 128] block holds chunks 2q (rows
        # 0:64) and 2q+1 (rows 64:128) -> xsT2 [128, (q, n)] via one copy.
        # The PSUM staging borrows a "val"-tag buffer.
        ps_x = psum.tile([128, 1024], F32, tag="ps", bufs=4)
        for q in range(4):
            nc.tensor.transpose(ps_x[:, q * 128:(q + 1) * 128],
                                xs4[:, q * 128:(q + 1) * 128], eye_sb[:])
        xsT2 = work.tile([128, 512], F16, tag="xsT", bufs=3)
        # DVE (not ACT) evacuates: ACT is the pacing engine at ~95% busy
        nc.vector.tensor_copy(xsT2[:], ps_x[:, 0:512])
        return xsT2

    def mm1_exp(xsT2, tp):
        # both parities' logits land in one 2-bank PSUM tile so a single
        # ACT exp covers 1024 elems (amortizes the ~352-cycle ACT fixed cost)
        ps_log = psum.tile([128, 1024], F32, tag="ps", bufs=4)
        for par in range(2):
            b = 64 * par
            nc.tensor.matmul(ps_log[:, 512 * par:512 * par + 512],
                             mt2h_sb[b:b + 64, tp * 128:(tp + 1) * 128],
                             xsT2[b:b + 64, :], start=True, stop=True)
        ex = work.tile([128, 1024], BF16, tag="exp", bufs=12)
        nc.scalar.activation(ex[:], ps_log[:],
                             mybir.ActivationFunctionType.Exp)
        return ex

    def chunk_mm2(exps, c):
        # chunk c lives at parity c%2, block c//2 of the exp tiles.
        # tp blocks land contiguously at 122+130*tp so the whole chunk's
        # normalize is ONE 4D-AP instruction (amortizes DVE/gpsimd fixed
        # cost) and the sums sit at a regular [p, 6, 2] stride pattern.
        q, par = divmod(c, 2)
        ps_val = psum.tile([128, 1024], F32, tag="ps", bufs=4)
        # base offset 122: block 2 ends exactly at col 512 and block 3
        # starts there, so no single matmul output straddles a PSUM bank
        # boundary (straddling corrupts a few boundary elements, timing-
        # dependent) while keeping one uniform 130-stride for the APs.
        for tp in range(6):
            nc.tensor.matmul(ps_val[:, 122 + 130 * tp:122 + 130 * tp + 130],
                             exps[tp][:, 512 * par + q * 128:
                                       512 * par + (q + 1) * 128],
                             mbd_sb[:, tp * 130:(tp + 1) * 130],
                             start=True, stop=True)
        sums_ap = (ps_val[:, 122:902].rearrange("p (a r) -> p a r", a=6)
                   [:, :, 128:130])
        rec = work.tile([128, 12], F32, tag="rec", bufs=8)
        nc.vector.reciprocal(
            rec[:].rearrange("p (a t) -> p a t", a=6), sums_ap)
        return ps_val, rec

    def _norm_aps(ps_or_vv, rec, vn, c, from_psum):
        if from_psum:
            in0 = (ps_or_vv[:, 122:902].rearrange("p (a r) -> p a r", a=6)
                   [:, :, 0:128].rearrange("p a (t d) -> p a t d", t=2))
        else:
            in0 = ps_or_vv[:].rearrange("p (a t d) -> p a t d", a=6, t=2)
        in1 = (rec[:].rearrange("p (a t) -> p a t", a=6)
               .unsqueeze(3).broadcast_to([128, 6, 2, D]))
        outp = (vn[:, c * 768:(c + 1) * 768]
                .rearrange("p (a t d) -> p a t d", a=6, t=2))
        return in0, in1, outp

    def chunk_norm(ps_val, rec, vn, c):
        in0, in1, outp = _norm_aps(ps_val, rec, vn, c, True)
        nc.vector.tensor_mul(outp, in0, in1)

    def chunk_norm_off(ps_val, rec, vn, c):
        # offloaded normalize: ACT evacuates PSUM -> bf16 SBUF in one copy
        # (unnormalized values can reach ~e^30: needs bf16 range), gpsimd
        # (otherwise idle) does the broadcast multiply, freeing DVE.
        vv = work.tile([128, 768], BF16, tag="vv", bufs=3)
        nc.scalar.copy(vv[:].rearrange("p (a r) -> p a r", a=6),
                       ps_val[:, 122:902].rearrange("p (a r) -> p a r", a=6)
                       [:, :, 0:128])
        in0, in1, outp = _norm_aps(vv, rec, vn, c, False)
        nc.gpsimd.tensor_mul(outp, in0, in1)

    def store(it, vn, half):
        # 0.75 MB half-stores: chunk k lives in vn column-group k (row
        # 8p+k), so half h = column-groups 4h..4h+3 = chunks 4h..4h+3.
        # Chunks 0-3 norm by slot 3, chunk 4 (with 5-7) by slot 4; issuing
        # each half as soon as it's ready smooths the DMA stream and trims
        # the epilogue tail. HBM side stays 6 KB-contiguous per partition.
        nc.sync.dma_start(
            out=out[1024 * it:1024 * it + 1024, :]
                .rearrange("(p c) f -> p c f", c=8)[:, 4 * half:4 * half + 4],
            in_=vn[:, 3072 * half:3072 * half + 3072]
                .rearrange("p (c f) -> p c f", c=4),
        )

    # -------- prologue: iteration 0's xsT, loads for 0 and 1 --------
    # iteration 0's load + tree + transpose chain is quartered (2 of the
    # 8 row-groups per step) so its tree starts ~3us after the first
    # quarter-DMA lands instead of waiting out the full 1.5 MB load; the
    # fill phase to the first store is ~1/3 of total runtime. fp16 (not
    # fp32) warm-up matmuls: an fp32 matmul in HIGH mode disables the
    # compiler's fast-weight-load for what follows, and the short burst
    # fits the PE-queue idle window before the first transpose arrives.
    xt0 = work.tile([128, 8 * L * D], F16, tag="xt", bufs=3)
    xt0v = xt0[:].rearrange("p (c f) -> p c f", c=8)
    xsrc = x[0:1024, :, :].rearrange("(p c) l d -> p c (l d)", c=8)
    for qt in range(4):
        nc.sync.dma_start(out=xt0v[:, 2 * qt:2 * qt + 2],
                          in_=xsrc[:, 2 * qt:2 * qt + 2])
    xts = {0: xt0}
    if NIT > 1:
        xts[1] = load(1)
    warm = psum.tile([128, 1024], F32, tag="ps", bufs=4)
    for _ in range(4):
        nc.tensor.matmul(warm[:, 0:512], mt2h_sb[0:64, 0:128],
                         mt2h_sb[0:64, 0:512], start=True, stop=True)
    t384_0 = work.tile([128, 8 * 384], F16, tag="t384", bufs=3)
    t192_0 = work.tile([128, 8 * 192], F16, tag="t192", bufs=2)
    xs2_0 = work.tile([128, 8 * 64], F16, tag="xs2", bufs=2)
    xs4_0 = work.tile([128, 8 * 64], F32, tag="xs4", bufs=2)
    ps_x0 = psum.tile([128, 1024], F32, tag="ps", bufs=4)
    for qt in range(4):
        sl = slice(2 * qt, 2 * qt + 2)
        xtv = xt0[:].rearrange("p (c h f) -> p c h f", c=8, h=2)[:, sl]
        nc.vector.tensor_add(
            t384_0[:].rearrange("p (c f) -> p c f", c=8)[:, sl],
            xtv[:, :, 0], xtv[:, :, 1])
        t384v = t384_0[:].rearrange("p (c h f) -> p c h f", c=8, h=2)[:, sl]
        nc.gpsimd.tensor_add(
            t192_0[:].rearrange("p (c f) -> p c f", c=8)[:, sl],
            t384v[:, :, 0], t384v[:, :, 1])
        t192v = t192_0[:].rearrange("p (c g f) -> p c g f", c=8, g=3)[:, sl]
        xs2v = xs2_0[:].rearrange("p (c f) -> p c f", c=8)[:, sl]
        nc.gpsimd.tensor_add(xs2v, t192v[:, :, 0], t192v[:, :, 1])
        nc.vector.tensor_add(
            xs4_0[:].rearrange("p (c f) -> p c f", c=8)[:, sl],
            xs2v, t192v[:, :, 2])
        nc.tensor.transpose(ps_x0[:, qt * 128:(qt + 1) * 128],
                            xs4_0[:, qt * 128:(qt + 1) * 128], eye_sb[:])
    xsT2 = work.tile([128, 512], F16, tag="xsT", bufs=3)
    nc.vector.tensor_copy(xsT2[:], ps_x0[:, 0:512])

    exps_prev = None
    vn_prev = None
    for it in range(NIT + 1):
        if it + 2 < NIT:
            xts[it + 2] = load(it + 2)
        exps = {}
        vn = None
        if it < NIT:
            vn = work.tile([128, 8 * T * D], F16, tag="vn", bufs=3)
        t384n = None
        xs4n = None
        xsT2_next = None
        # interleave this iteration's mm1/exp pairs with the previous
        # iteration's mm2 chunks so PE never idles on ACT's exp pace; the
        # next iteration's tree and transposes are woven in mid-body.
        # chunk schedule: all 8 chunks spread across the 6 tp slots so
        # every engine stays co-busy through the body (a dedicated chunk
        # tail was tried and regressed 15%: it serializes engine phases).
        # gpsimd-offloaded chunks (5-7) pair with DVE chunks mid-body so
        # their ACT evacuation copies land between exps.
        slot_chunks = {0: [(0, False)], 1: [(5, True), (1, False)],
                       2: [(6, True), (2, False)], 3: [(7, True), (3, False)],
                       4: [(4, False)], 5: []}
        for tp in range(6):
            # previous iteration's chunks first: their inputs are always
            # ready, so they hide the mm1->exp ps_log wait and keep PE
            # dense (HAM stays at full clock).
            work_items = []
            if it > 0:
                for c, off in slot_chunks[tp]:
                    pv, rec = chunk_mm2(exps_prev, c)
                    work_items.append((c, off, pv, rec))
            if it < NIT:
                exps[tp] = mm1_exp(xsT2, tp)
            for c, off, pv, rec in work_items:
                if off:
                    chunk_norm_off(pv, rec, vn_prev, c)
                else:
                    chunk_norm(pv, rec, vn_prev, c)
            if tp == 0 and it + 1 < NIT:
                t384n = tree_l1(xts.pop(it + 1))
            if tp == 1 and t384n is not None:
                xs4n = tree_rest(t384n)
            if tp == 3 and it > 0:
                store(it - 1, vn_prev, 0)
            if tp == 4 and it > 0:
                store(it - 1, vn_prev, 1)
            if tp == 5 and xs4n is not None:
                xsT2_next = build_xsT(xs4n)
        if xsT2_next is not None:
            xsT2 = xsT2_next
        exps_prev, vn_prev = exps, vn


_NC_CACHE = {}


def build_nc():
    if "nc" in _NC_CACHE:
        return _NC_CACHE["nc"]
    nc = bacc.Bacc("TRN2", target_bir_lowering=False, debug=False,
                   num_devices=NCORES)
    # x is pre-transposed on the host to [BS, N, L, D], n-padded to 896 rows
    # per batch with zeros, flattened to [7168, 12, 64] and cast fp16. The
    # output is produced padded as [7168, (t d)] fp16; the host slices off
    # the 13 pad rows per batch and upcasts.
    x_ap = nc.dram_tensor("x_sh", [ROWS, L, D], F16, kind="ExternalInput").ap()
    mt2h_ap = nc.dram_tensor("mt2h", [128, 6 * 128], F16, kind="ExternalInput").ap()
    mbd_ap = nc.dram_tensor("mbd", [128, 6 * 130], BF16, kind="ExternalInput").ap()
    eye_ap = nc.dram_tensor("eye", [128, 128], F32, kind="ExternalInput").ap()
    out_ap = nc.dram_tensor("out", [ROWS, T * D], F16, kind="ExternalOutput").ap()
    with tile.TileContext(nc) as tc:
        kernel_body(tc, out_ap, x_ap, mt2h_ap, mbd_ap, eye_ap)
    nc.compile()
    _NC_CACHE["nc"] = nc
    return nc


def make_in_maps(x, M):
    import ml_dtypes
    x = np.asarray(x, dtype=np.float32)
    mt2h, mbd, eye = build_consts(M)
    mbd_bf = mbd.astype(ml_dtypes.bfloat16)
    maps = []
    for i in range(NCORES):
        xp = np.zeros((BS, NPAD, L, D), np.float16)
        xp[:, :N] = x[i * BS:(i + 1) * BS].transpose(0, 2, 1, 3).astype(np.float16)
        maps.append({"x_sh": xp.reshape(ROWS, L, D),
                     "mt2h": mt2h, "mbd": mbd_bf, "eye": eye})
    return maps


def gather_outputs(res):
    outs = []
    for i in range(NCORES):
        o = np.asarray(res[i]["out"], dtype=np.float32)
        o = o.reshape(BS, NPAD, T, D)[:, :N].transpose(0, 2, 1, 3)
        outs.append(o)
    return np.ascontiguousarray(np.concatenate(outs, axis=0))


def kernel(x, M):
    nc = build_nc()
    in_maps = make_in_maps(x, M)
    res = run_bass_kernel_spmd(nc, in_maps, list(range(NCORES))).results
    return gather_outputs(res)


if __name__ == "__main__":
    rng = np.random.default_rng(0)
    x = rng.standard_normal((B, L, N, D), dtype=np.float32)
    M = (rng.standard_normal((T, MNUM, D), dtype=np.float32) * 0.125).astype(np.float32)
    out = kernel(x, M)
    print("out", out.shape, out.dtype, float(np.abs(out).max()))



# revision 31
# speedup vs baseline: 1.1784x; 1.0219x over previous
"""Trainium2 Bass kernel for nn_MemoryAugmented (scatter_memory).

Computes, for full inputs x:[64,12,883,64], M:[12,64,64]:
    score = softmax(einsum('blnd,tmd->btnm', x, M), axis=-1)
    out   = einsum('btnm,tmd->btnd', score, M)

Distribution: data-parallel over batch across 8 NeuronCores (8 batches
per core); the memory bank M is replicated, shipped pre-transformed into
two constant matrices (paired-t M^T for mm1, block-diagonal M + ones
columns for mm2's fused row sums).

Precision: x and M travel as fp16 (matmuls run at 1 cycle/row vs 4 for
fp32, HBM traffic halves); exp values are bf16 (need fp32-like range);
PSUM accumulation is always fp32; output is stored fp16 and upcast on
the host. Measured end-to-end max rel err ~4e-3 vs the 2e-2 gate.

Per-core dataflow, 7 iterations of 1024 rows r = (b, n), software-
pipelined one deep so no engine waits on another's latest result.
Each body iterates 6 t-pair slots, each issuing: one or two 128-row
value chunks of mm2(it-1) + reciprocal + normalize (5 chunks on DVE,
3 via ACT-evacuate + gpsimd), then mm1(it) x2 + one merged exp(it).
The l-sum tree for it+1 (L1/L4 on DVE at fp16 2x, L2/L3 on gpsimd),
the two 0.75 MB store halves of it-1, and the PE transposes + DVE
copy building xsT(it+1) are woven into fixed slots. All PSUM rides
one shared 4-buffer pool of [128,1024] tiles (8 banks); no matmul
output ever crosses a 2 KB PSUM bank boundary (silent corruption).
"""
import sys

for _p in ("/opt/trn_rl_repo",):
    if _p not in sys.path:
        sys.path.insert(0, _p)

from contextlib import ExitStack

import numpy as np

import concourse.bass as bass
import concourse.bacc as bacc
import concourse.tile as tile
from concourse import mybir
from concourse._compat import with_exitstack
from concourse.bass_utils import run_bass_kernel_spmd

B, L, N, D = 64, 12, 883, 64
T, MNUM = 12, 64
NCORES = 8
BS = B // NCORES          # 8 batches per core
NPAD = 896                # per-batch row pad (7*128)
ROWS = BS * NPAD          # 7168 rows per core
NIT = 7                   # iterations of 1024 rows
F32 = mybir.dt.float32
F16 = mybir.dt.float16
BF16 = mybir.dt.bfloat16


def build_consts(M):
    """Host-side layout prep (pure data movement) of the memory bank."""
    M = np.asarray(M, dtype=np.float32)
    mt2h = np.zeros((64, 6 * 128), np.float16)   # [d, (tp, q, m)] = M[2tp+q].T
    mbd = np.zeros((128, 6 * 130), np.float32)   # [(q, m), (tp, q, d | sums)]
    for tp in range(6):
        t0, t1 = 2 * tp, 2 * tp + 1
        mt2h[:, tp * 128 + 0:tp * 128 + 64] = M[t0].T.astype(np.float16)
        mt2h[:, tp * 128 + 64:tp * 128 + 128] = M[t1].T.astype(np.float16)
        mbd[0:64, tp * 130 + 0:tp * 130 + 64] = M[t0]
        mbd[64:128, tp * 130 + 64:tp * 130 + 128] = M[t1]
        mbd[0:64, tp * 130 + 128] = 1.0
        mbd[64:128, tp * 130 + 129] = 1.0
    # mirrored into both partition halves: parity-1 matmuls read their
    # stationary from partitions 64:128 (row group h1)
    mt2h2 = np.concatenate([mt2h, mt2h], axis=0)
    eye = np.eye(128, dtype=np.float32)
    return mt2h2, mbd, eye


@with_exitstack
def kernel_body(ctx: ExitStack, tc: "tile.TileContext", out: bass.AP,
                x: bass.AP, mt2h: bass.AP, mbd: bass.AP, eye: bass.AP):
    nc = tc.nc
    consts = ctx.enter_context(tc.tile_pool(name="consts", bufs=1))
    work = ctx.enter_context(tc.tile_pool(name="work", bufs=2))
    psum = ctx.enter_context(tc.tile_pool(name="psum", bufs=1, space="PSUM"))

    # const loads ride the scalar HWDGE ring (idle at kernel start) so the
    # first x-load isn't queued behind them on the sync ring's FIFO.
    mt2h_sb = consts.tile([128, 6 * 128], F16)
    nc.scalar.dma_start(out=mt2h_sb[:], in_=mt2h[:])
    mbd_sb = consts.tile([128, 6 * 130], BF16)
    nc.scalar.dma_start(out=mbd_sb[:], in_=mbd[:])
    eye_sb = consts.tile([128, 128], F32)
    nc.scalar.dma_start(out=eye_sb[:], in_=eye[:])

    def load(it):
        # one 1.5 MB load; partition p <- rows 8p..8p+7 (12 KB contiguous)
        xt = work.tile([128, 8 * L * D], F16, tag="xt", bufs=3)
        nc.sync.dma_start(
            out=xt[:].rearrange("p (c f) -> p c f", c=8),
            in_=x[1024 * it:1024 * it + 1024, :, :]
                .rearrange("(p c) l d -> p c (l d)", c=8),
        )
        return xt

    def tree_l1(xt):
        # 12 -> 6 on DVE: fp16 tensor_tensor runs 2x there (~1.8us) vs
        # gpsimd's measured ~0.5 elem/cycle (~6us for the same add)
        t384 = work.tile([128, 8 * 384], F16, tag="t384", bufs=3)
        xtv = xt[:].rearrange("p (c h f) -> p c h f", c=8, h=2)
        nc.vector.tensor_add(t384[:].rearrange("p (c f) -> p c f", c=8),
                             xtv[:, :, 0], xtv[:, :, 1])
        return t384

    def tree_rest(t384):
        # L2/L3 on gpsimd, L4 on DVE (DVE is the busier engine)
        t192 = work.tile([128, 8 * 192], F16, tag="t192", bufs=2)
        t384v = t384[:].rearrange("p (c h f) -> p c h f", c=8, h=2)
        nc.gpsimd.tensor_add(t192[:].rearrange("p (c f) -> p c f", c=8),
                             t384v[:, :, 0], t384v[:, :, 1])
        t192v = t192[:].rearrange("p (c g f) -> p c g f", c=8, g=3)
        xs2 = work.tile([128, 8 * 64], F16, tag="xs2", bufs=2)
        xs2v = xs2[:].rearrange("p (c f) -> p c f", c=8)
        nc.gpsimd.tensor_add(xs2v, t192v[:, :, 0], t192v[:, :, 1])
        # fp32 to match the PE transpose's fp32 "val" PSUM staging tile
        xs4 = work.tile([128, 8 * 64], F32, tag="xs4", bufs=2)
        nc.vector.tensor_add(xs4[:].rearrange("p (c f) -> p c f", c=8),
                             xs2v, t192v[:, :, 2])
        return xs4

    def build_xsT(xs4):
        # paired transpose: each [128, 128] block holds chunks 2q (rows
        # 0:64) and 2q+1 (rows 64:128) -> xsT2 [128, (q, n)] via one copy.
        # The PSUM staging borrows a "val"-tag buffer.
        ps_x = psum.tile([128, 1024], F32, tag="ps", bufs=4)
        for q in range(4):
            nc.tensor.transpose(ps_x[:, q * 128:(q + 1) * 128],
                                xs4[:, q * 128:(q + 1) * 128], eye_sb[:])
        xsT2 = work.tile([128, 512], F16, tag="xsT", bufs=3)
        # DVE (not ACT) evacuates: ACT is the pacing engine at ~95% busy
        nc.vector.tensor_copy(xsT2[:], ps_x[:, 0:512])
        return xsT2

    def mm1_exp(xsT2, tp):
        # both parities' logits land in one 2-bank PSUM tile so a single
        # ACT exp covers 1024 elems (amortizes the ~352-cycle ACT fixed cost)
        ps_log = psum.tile([128, 1024], F32, tag="ps", bufs=4)
        for par in range(2):
            b = 64 * par
            nc.tensor.matmul(ps_log[:, 512 * par:512 * par + 512],
                             mt2h_sb[b:b + 64, tp * 128:(tp + 1) * 128],
                             xsT2[b:b + 64, :], start=True, stop=True)
        ex = work.tile([128, 1024], BF16, tag="exp", bufs=12)
        nc.scalar.activation(ex[:], ps_log[:],
                             mybir.ActivationFunctionType.Exp)
        return ex

    def chunk_mm2(exps, c):
        # chunk c lives at parity c%2, block c//2 of the exp tiles.
        # tp blocks land contiguously at 122+130*tp so the whole chunk's
        # normalize is ONE 4D-AP instruction (amortizes DVE/gpsimd fixed
        # cost) and the sums sit at a regular [p, 6, 2] stride pattern.
        q, par = divmod(c, 2)
        ps_val = psum.tile([128, 1024], F32, tag="ps", bufs=4)
        # heartbeat into dead columns (chunks use 122:902): a cheap fp16
        # matmul on chunks 0/4 nudges the PE HAM activity monitor away
        # from re-throttling the array clock to 1.2 GHz.
        if c in (0, 4):
            nc.tensor.matmul(ps_val[:, 0:122], mt2h_sb[0:64, 0:128],
                             mt2h_sb[0:64, 0:122], start=True, stop=True)
        # base offset 122: block 2 ends exactly at col 512 and block 3
        # starts there, so no single matmul output straddles a PSUM bank
        # boundary (straddling corrupts a few boundary elements, timing-
        # dependent) while keeping one uniform 130-stride for the APs.
        for tp in range(6):
            nc.tensor.matmul(ps_val[:, 122 + 130 * tp:122 + 130 * tp + 130],
                             exps[tp][:, 512 * par + q * 128:
                                       512 * par + (q + 1) * 128],
                             mbd_sb[:, tp * 130:(tp + 1) * 130],
                             start=True, stop=True)
        sums_ap = (ps_val[:, 122:902].rearrange("p (a r) -> p a r", a=6)
                   [:, :, 128:130])
        rec = work.tile([128, 12], F32, tag="rec", bufs=8)
        nc.vector.reciprocal(
            rec[:].rearrange("p (a t) -> p a t", a=6), sums_ap)
        return ps_val, rec

    def _norm_aps(ps_or_vv, rec, vn, c, from_psum):
        if from_psum:
            in0 = (ps_or_vv[:, 122:902].rearrange("p (a r) -> p a r", a=6)
                   [:, :, 0:128].rearrange("p a (t d) -> p a t d", t=2))
        else:
            in0 = ps_or_vv[:].rearrange("p (a t d) -> p a t d", a=6, t=2)
        in1 = (rec[:].rearrange("p (a t) -> p a t", a=6)
               .unsqueeze(3).broadcast_to([128, 6, 2, D]))
        outp = (vn[:, c * 768:(c + 1) * 768]
                .rearrange("p (a t d) -> p a t d", a=6, t=2))
        return in0, in1, outp

    def chunk_norm(ps_val, rec, vn, c):
        in0, in1, outp = _norm_aps(ps_val, rec, vn, c, True)
        nc.vector.tensor_mul(outp, in0, in1)

    def chunk_norm_off(ps_val, rec, vn, c):
        # offloaded normalize: ACT evacuates PSUM -> bf16 SBUF in one copy
        # (unnormalized values can reach ~e^30: needs bf16 range), gpsimd
        # (otherwise idle) does the broadcast multiply, freeing DVE.
        vv = work.tile([128, 768], BF16, tag="vv", bufs=3)
        nc.scalar.copy(vv[:].rearrange("p (a r) -> p a r", a=6),
                       ps_val[:, 122:902].rearrange("p (a r) -> p a r", a=6)
                       [:, :, 0:128])
        in0, in1, outp = _norm_aps(vv, rec, vn, c, False)
        nc.gpsimd.tensor_mul(outp, in0, in1)

    def store(it, vn, half):
        # 0.75 MB half-stores: chunk k lives in vn column-group k (row
        # 8p+k), so half h = column-groups 4h..4h+3 = chunks 4h..4h+3.
        # Chunks 0-3 norm by slot 3, chunk 4 (with 5-7) by slot 4; issuing
        # each half as soon as it's ready smooths the DMA stream and trims
        # the epilogue tail. HBM side stays 6 KB-contiguous per partition.
        nc.sync.dma_start(
            out=out[1024 * it:1024 * it + 1024, :]
                .rearrange("(p c) f -> p c f", c=8)[:, 4 * half:4 * half + 4],
            in_=vn[:, 3072 * half:3072 * half + 3072]
                .rearrange("p (c f) -> p c f", c=4),
        )

    # -------- prologue: iteration 0's xsT, loads for 0 and 1 --------
    # iteration 0's load + tree + transpose chain is quartered (2 of the
    # 8 row-groups per step) so its tree starts ~3us after the first
    # quarter-DMA lands instead of waiting out the full 1.5 MB load; the
    # fill phase to the first store is ~1/3 of total runtime. fp16 (not
    # fp32) warm-up matmuls: an fp32 matmul in HIGH mode disables the
    # compiler's fast-weight-load for what follows, and the short burst
    # fits the PE-queue idle window before the first transpose arrives.
    xt0 = work.tile([128, 8 * L * D], F16, tag="xt", bufs=3)
    xt0v = xt0[:].rearrange("p (c f) -> p c f", c=8)
    xsrc = x[0:1024, :, :].rearrange("(p c) l d -> p c (l d)", c=8)
    for qt in range(4):
        nc.sync.dma_start(out=xt0v[:, 2 * qt:2 * qt + 2],
                          in_=xsrc[:, 2 * qt:2 * qt + 2])
    xts = {0: xt0}
    if NIT > 1:
        xts[1] = load(1)
    warm = psum.tile([128, 1024], F32, tag="ps", bufs=4)
    for _ in range(4):
        nc.tensor.matmul(warm[:, 0:512], mt2h_sb[0:64, 0:128],
                         mt2h_sb[0:64, 0:512], start=True, stop=True)
    t384_0 = work.tile([128, 8 * 384], F16, tag="t384", bufs=3)
    t192_0 = work.tile([128, 8 * 192], F16, tag="t192", bufs=2)
    xs2_0 = work.tile([128, 8 * 64], F16, tag="xs2", bufs=2)
    xs4_0 = work.tile([128, 8 * 64], F32, tag="xs4", bufs=2)
    ps_x0 = psum.tile([128, 1024], F32, tag="ps", bufs=4)
    for qt in range(4):
        sl = slice(2 * qt, 2 * qt + 2)
        xtv = xt0[:].rearrange("p (c h f) -> p c h f", c=8, h=2)[:, sl]
        nc.vector.tensor_add(
            t384_0[:].rearrange("p (c f) -> p c f", c=8)[:, sl],
            xtv[:, :, 0], xtv[:, :, 1])
        t384v = t384_0[:].rearrange("p (c h f) -> p c h f", c=8, h=2)[:, sl]
        nc.gpsimd.tensor_add(
            t192_0[:].rearrange("p (c f) -> p c f", c=8)[:, sl],
            t384v[:, :, 0], t384v[:, :, 1])
        t192v = t192_0[:].rearrange("p (c g f) -> p c g f", c=8, g=3)[:, sl]
        xs2v = xs2_0[:].rearrange("p (c f) -> p c f", c=8)[:, sl]
        nc.gpsimd.tensor_add(xs2v, t192v[:, :, 0], t192v[:, :, 1])
        nc.vector.tensor_add(
            xs4_0[:].rearrange("p (c f) -> p c f", c=8)[:, sl],
            xs2v, t192v[:, :, 2])
        nc.tensor.transpose(ps_x0[:, qt * 128:(qt + 1) * 128],
                            xs4_0[:, qt * 128:(qt + 1) * 128], eye_sb[:])
    xsT2 = work.tile([128, 512], F16, tag="xsT", bufs=3)
    nc.vector.tensor_copy(xsT2[:], ps_x0[:, 0:512])

    exps_prev = None
    vn_prev = None
    for it in range(NIT + 1):
        if it + 2 < NIT:
            xts[it + 2] = load(it + 2)
        exps = {}
        vn = None
        if it < NIT:
            vn = work.tile([128, 8 * T * D], F16, tag="vn", bufs=3)
        t384n = None
        xs4n = None
        xsT2_next = None
        # interleave this iteration's mm1/exp pairs with the previous
        # iteration's mm2 chunks so PE never idles on ACT's exp pace; the
        # next iteration's tree and transposes are woven in mid-body.
        # chunk schedule: all 8 chunks spread across the 6 tp slots so
        # every engine stays co-busy through the body (a dedicated chunk
        # tail was tried and regressed 15%: it serializes engine phases).
        # gpsimd-offloaded chunks (5-7) pair with DVE chunks mid-body so
        # their ACT evacuation copies land between exps.
        slot_chunks = {0: [(0, False)], 1: [(5, True), (1, False)],
                       2: [(6, True), (2, False)], 3: [(7, True), (3, False)],
                       4: [(4, False)], 5: []}
        for tp in range(6):
            # previous iteration's chunks first: their inputs are always
            # ready, so they hide the mm1->exp ps_log wait and keep PE
            # dense (HAM stays at full clock).
            work_items = []
            if it > 0:
                for c, off in slot_chunks[tp]:
                    pv, rec = chunk_mm2(exps_prev, c)
                    work_items.append((c, off, pv, rec))
            if it < NIT:
                exps[tp] = mm1_exp(xsT2, tp)
            for c, off, pv, rec in work_items:
                if off:
                    chunk_norm_off(pv, rec, vn_prev, c)
                else:
                    chunk_norm(pv, rec, vn_prev, c)
            if tp == 0 and it + 1 < NIT:
                t384n = tree_l1(xts.pop(it + 1))
            if tp == 1 and t384n is not None:
                xs4n = tree_rest(t384n)
            if tp == 3 and it > 0:
                store(it - 1, vn_prev, 0)
            if tp == 4 and it > 0:
                store(it - 1, vn_prev, 1)
            if tp == 5 and xs4n is not None:
                xsT2_next = build_xsT(xs4n)
        if xsT2_next is not None:
            xsT2 = xsT2_next
        exps_prev, vn_prev = exps, vn


_NC_CACHE = {}


def build_nc():
    if "nc" in _NC_CACHE:
        return _NC_CACHE["nc"]
    nc = bacc.Bacc("TRN2", target_bir_lowering=False, debug=False,
                   num_devices=NCORES)
    # x is pre-transposed on the host to [BS, N, L, D], n-padded to 896 rows
    # per batch with zeros, flattened to [7168, 12, 64] and cast fp16. The
    # output is produced padded as [7168, (t d)] fp16; the host slices off
    # the 13 pad rows per batch and upcasts.
    x_ap = nc.dram_tensor("x_sh", [ROWS, L, D], F16, kind="ExternalInput").ap()
    mt2h_ap = nc.dram_tensor("mt2h", [128, 6 * 128], F16, kind="ExternalInput").ap()
    mbd_ap = nc.dram_tensor("mbd", [128, 6 * 130], BF16, kind="ExternalInput").ap()
    eye_ap = nc.dram_tensor("eye", [128, 128], F32, kind="ExternalInput").ap()
    out_ap = nc.dram_tensor("out", [ROWS, T * D], F16, kind="ExternalOutput").ap()
    with tile.TileContext(nc) as tc:
        kernel_body(tc, out_ap, x_ap, mt2h_ap, mbd_ap, eye_ap)
    nc.compile()
    _NC_CACHE["nc"] = nc
    return nc


def make_in_maps(x, M):
    import ml_dtypes
    x = np.asarray(x, dtype=np.float32)
    mt2h, mbd, eye = build_consts(M)
    mbd_bf = mbd.astype(ml_dtypes.bfloat16)
    maps = []
    for i in range(NCORES):
        xp = np.zeros((BS, NPAD, L, D), np.float16)
        xp[:, :N] = x[i * BS:(i + 1) * BS].transpose(0, 2, 1, 3).astype(np.float16)
        maps.append({"x_sh": xp.reshape(ROWS, L, D),
                     "mt2h": mt2h, "mbd": mbd_bf, "eye": eye})
    return maps


def gather_outputs(res):
    outs = []
    for i in range(NCORES):
        o = np.asarray(res[i]["out"], dtype=np.float32)
        o = o.reshape(BS, NPAD, T, D)[:, :N].transpose(0, 2, 1, 3)
        outs.append(o)
    return np.ascontiguousarray(np.concatenate(outs, axis=0))


def kernel(x, M):
    nc = build_nc()
    in_maps = make_in_maps(x, M)
    res = run_bass_kernel_spmd(nc, in_maps, list(range(NCORES))).results
    return gather_outputs(res)


if __name__ == "__main__":
    rng = np.random.default_rng(0)
    x = rng.standard_normal((B, L, N, D), dtype=np.float32)
    M = (rng.standard_normal((T, MNUM, D), dtype=np.float32) * 0.125).astype(np.float32)
    out = kernel(x, M)
    print("out", out.shape, out.dtype, float(np.abs(out).max()))

